# revision 9
# baseline (speedup 1.0000x reference)
"""Trainium2 Bass kernel for a 3-layer binary-weight MLP.

Problem (nn_MLP_56779467653689):
    x: [8192, 1024] f32
    h = relu(s0 * (x @ W0)) * 2      W0 = 2*k0-1  in {-1,+1}, [1024, 4096]
    h = relu(s1 * (h @ W1)) * 2      W1 [4096, 4096]
    out = s2 * (h @ W2)              W2 [4096, 1024]

Strategy: data-parallel over tokens across 8 NeuronCores (1024/core),
activations [features, tokens] in SBUF.

Layer 1 is hybrid-precision: the first 2048 contraction features run one
level of Winograd-Strassen in bf16 (7/8 multiplies); the other 2048 run
naive fp8(e4m3) matmuls in DoubleRow perf mode (2 contraction rows per PE
cell per cycle).  The fp8 activations are mean-removed (hardcoded mu,
exactly compensated by a per-output-feature bias at eviction) to shrink
quantization error.  The fp8 partial sums accumulate directly into the
Winograd chains' PSUM banks using host-precomputed weight variants
(c11:+Wg, c12:Wg-W16g, c21:-W16g, c22:+W16g, all exact in e4m3), so the
Winograd combine algebra routes them to the right output quadrant with no
extra vector work.

Layer 2 uses one level of bf16 Winograd-Strassen over the full
contraction (fp8 there would blow the error budget).  Layer 0 is naive
bf16.  PE multiply count: L0 8/8, L1 (7/8)/2 + (1/2)/2-rate, L2 7/8.
"""

from contextlib import ExitStack

import ml_dtypes
import numpy as np

P = 128
TOKENS = 8192
D_IN = 1024
D_H = 4096
D_OUT = 1024
N_CORES = 8
TOK_PER_CORE = TOKENS // N_CORES  # 1024
TOK_TILE = 512
NT = TOK_PER_CORE // TOK_TILE  # 2

MU1 = 0.7975461096539505  # approx mean of h1; any value is corrected exactly

BF16 = ml_dtypes.bfloat16
E4M3 = ml_dtypes.float8_e4m3

TRACE = False
TRACE_CORES = None
LAST_EXEC_TIME_NS = None
LAST_RESULT = None

_cache = {}


def _prune_dma_waits(nc, max_waits=1):
    """Drop transitively-implied waits from DMA instructions.

    DMA queue-entry descriptors hold a single sync wait; Tile's sem
    assignment is per-proc minimal but not transitively minimal across
    procs, so a recycled SBUF slot's DMA can carry WAR (engine) + WAW
    (prev slot writer's DMA lane) + lane-recycle waits = 3. The WAW (and
    often the recycle) wait is implied by the engine wait: the readers
    counted by the WAR threshold themselves waited on those DMAs.

    Soundness: a wait (s >= v) on instruction I is dropped only when the
    completion clocks implied by I's *other* waits already guarantee
    cumulative increments of s reached v. Completion clocks are built
    forward over the scheduled BIR order giving same-stream predecessor
    credit only to in-order engines (PE/ACT/DVE/SP), never to DMA lanes
    or Pool. Unrecognized wait/update modes contribute no credit, so
    unknowns can only inhibit pruning, never enable it.
    """
    import bisect

    import bass_rust

    IN_ORDER_ENGINES = {
        "EngineType.PE",
        "EngineType.Activation",
        "EngineType.DVE",
        "EngineType.SP",
    }

    sem_hist = {}
    sem_cum = {}
    eng_clock = {}
    poisoned = set()

    def cc(sem, val):
        if sem in poisoned:
            return None
        hist = sem_hist.get(sem)
        if not hist or hist[0][-1] < val:
            return None
        return hist[1][bisect.bisect_left(hist[0], val)]

    def merge(dst, src):
        for k, v in src.items():
            if dst.get(k, 0) < v:
                dst[k] = v

    pruned = 0
    for bb in nc.m.functions[0].blocks:
        for inst in bb.instructions:
            si = inst.sync_info
            waits = list(si.on_wait or []) if si is not None else []
            ups = list(si.on_update or []) if si is not None else []
            is_dma = type(inst).__name__ == "InstDMACopy"

            clock = {}
            if not is_dma:
                prev = eng_clock.get(str(inst.engine))
                if prev is not None and str(inst.engine) in IN_ORDER_ENGINES:
                    merge(clock, prev)
            for w in waits:
                if w.wait_mode == "sem-ge-imm" and w.wait_value is not None:
                    c = cc(w.ant_name, w.wait_value)
                    if c is not None:
                        merge(clock, c)

            tname = type(inst).__name__
            if is_dma:
                cap = max_waits
            elif tname in ("InstDrain", "InstEventSemaphore", "InstCall",
                           "InstUnconditionalBranch", "InstISA"):
                cap = None
            else:
                cap = 2

            if cap is not None and len(waits) > cap:
                kept = list(waits)
                changed = True
                while len(kept) > cap and changed:
                    changed = False
                    for w in list(kept):
                        if w.wait_mode != "sem-ge-imm" or w.wait_value is None:
                            continue
                        implied = {}
                        provable = True
                        for o in kept:
                            if o is w:
                                continue
                            if o.wait_mode != "sem-ge-imm" or o.wait_value is None:
                                provable = False
                                break
                            c = cc(o.ant_name, o.wait_value)
                            if c is None:
                                provable = False
                                break
                            merge(implied, c)
                        if provable and implied.get(w.ant_name, 0) >= w.wait_value:
                            kept.remove(w)
                            pruned += 1
                            changed = True
                            break
                if len(kept) != len(waits):
                    inst.sync_info = bass_rust.SyncInfo(on_wait=kept, on_update=ups)

            own = {}
            for u in ups:
                if u.update_mode not in ("sem-inc", "sem-add-imm"):
                    poisoned.add(u.ant_name)
                    continue
                inc = 1 if u.update_mode == "sem-inc" else u.update_value
                if inc is None:
                    poisoned.add(u.ant_name)
                    continue
                sem = u.ant_name
                sem_cum[sem] = sem_cum.get(sem, 0) + inc
                own[sem] = sem_cum[sem]
            merge(clock, own)
            for sem, cum in own.items():
                vals, clocks = sem_hist.setdefault(sem, ([], []))
                vals.append(cum)
                clocks.append(clock)
            if not is_dma:
                eng_clock[str(inst.engine)] = clock
    return pruned


def _build(a0, a1):
    """SPMD single-core program. L2's eviction scale is folded into its
    host-packed Strassen combos; a0/a1 are applied at eviction."""
    import concourse.mybir as mybir
    import concourse.tile as tile
    from concourse import bacc

    nc = bacc.Bacc(
        "TRN2",
        target_bir_lowering=False,
        debug=False,
        enable_asserts=False,
        num_devices=N_CORES,
    )
    bf = mybir.dt.bfloat16
    f32 = mybir.dt.float32
    e4 = mybir.dt.float8e4
    DR = mybir.MatmulPerfMode.DoubleRow

    xt = nc.dram_tensor("xt", [NT, P, D_IN // P * TOK_TILE], bf,
                        kind="ExternalInput")
    w0p = nc.dram_tensor("w0p", [D_H // P, P, D_IN], bf, kind="ExternalInput")
    w1w = nc.dram_tensor("w1w", [7, 16, P, 1024], bf, kind="ExternalInput")
    w1f = nc.dram_tensor("w1f", [4, 16, 8, P, 2, P], e4, kind="ExternalInput")
    b1t = nc.dram_tensor("b1t", [P, 33], f32, kind="ExternalInput")
    w2s = nc.dram_tensor("w2s", [7, 4, P, D_H // 2], bf, kind="ExternalInput")
    outt = nc.dram_tensor("outt", [D_OUT, TOK_PER_CORE], bf, kind="ExternalOutput")

    relu = mybir.ActivationFunctionType.Relu
    ident = mybir.ActivationFunctionType.Identity

    with tile.TileContext(nc) as tc, ExitStack() as ctx:
        flat = ctx.enter_context(tc.tile_pool(name="flat", bufs=1))
        wpool = ctx.enter_context(tc.tile_pool(name="wp", bufs=8))
        fwpool = ctx.enter_context(tc.tile_pool(name="fwp", bufs=8))
        upool = ctx.enter_context(tc.tile_pool(name="up", bufs=1))
        mpool = ctx.enter_context(tc.tile_pool(name="mp", bufs=2))
        spool = ctx.enter_context(tc.tile_pool(name="sp", bufs=4))
        opool = ctx.enter_context(tc.tile_pool(name="op", bufs=4))
        pspool = ctx.enter_context(tc.tile_pool(name="psp", bufs=8, space="PSUM"))

        def ftile(tag):
            return flat.tile([P, TOK_TILE], bf, tag=tag, name=tag)

        # bias vector for L1 eviction (per out-feature, column n = strip n);
        # column 32 holds -mu1 for the fp8 mean-removal cast.
        b1 = flat.tile([P, 33], f32, tag="b1", name="b1")
        nc.sync.dma_start(out=b1[:], in_=b1t[:, :])

        # ---- layer 0: naive. x is host-packed to [t, 128, j*512+c] so each
        # half loads as wide DMAs split across queues.
        xbig = []
        for t in range(NT):
            xb = flat.tile([P, D_IN // P * TOK_TILE], bf, tag=f"xb_{t}",
                           name=f"xb_{t}")
            for j in range(D_IN // P):
                q = nc.scalar if j % 2 == 0 else nc.gpsimd
                q.dma_start(
                    out=xb[:, j * TOK_TILE : (j + 1) * TOK_TILE],
                    in_=xt[t, :, j * TOK_TILE : (j + 1) * TOK_TILE],
                )
            xbig.append(xb)

        def x_sl(j, t):
            return xbig[t][:, j * TOK_TILE : (j + 1) * TOK_TILE]

        h1 = {}
        w0tiles = {}
        # fp8 copies of h1 strips 16..31 (mean-removed), paired for DoubleRow
        f8 = {}
        for pr in range(8):
            for t in range(NT):
                f8[(pr, t)] = flat.tile([P, 2, TOK_TILE], e4,
                                        tag=f"f8_{pr}_{t}", name=f"f8_{pr}_{t}")

        def l0_chain(n, t):
            if n not in w0tiles:
                w = wpool.tile([P, D_IN], bf, tag="w", name=f"w0_{n}")
                nc.sync.dma_start(out=w[:], in_=w0p[n])
                w0tiles[n] = w
            w = w0tiles[n]
            ps = pspool.tile([P, TOK_TILE], f32, tag="ps", name=f"ps0_{n}_{t}")
            for j in range(D_IN // P):
                nc.tensor.matmul(
                    ps[:],
                    w[:, j * P : (j + 1) * P],
                    x_sl(j, t),
                    start=(j == 0),
                    stop=(j == D_IN // P - 1),
                )
            ht = ftile(f"h1_{n}_{t}")
            nc.scalar.activation(ht[:], ps[:], relu, scale=a0)
            h1[(n, t)] = ht
            if n >= 16:
                # fp8 copy for L1's DoubleRow half: (h - mu) cast to e4m3
                s = n - 16
                nc.scalar.activation(
                    f8[(s // 2, t)][:, s % 2, :], ht[:], ident,
                    bias=b1[:, 32:33],
                )

        # First strips run token-half 0 only, buying arrival slack for the
        # second half of x; then the normal (n, t) sweep.
        l0_order = [(n, 0) for n in range(4)] + [(n, 1) for n in range(4)]
        l0_order += [(n, t) for n in range(4, D_H // P) for t in range(NT)]
        for n, t in l0_order:
            l0_chain(n, t)

        # ---- L1 prep: T-combos for the bf16-Winograd half (strips 0..15),
        # written into the now-dead strips 16..31 slots. All reads are from
        # strips 0..15, so they never race the fp8 casts above.
        t1h, t2h, t3h, t4h = {}, {}, {}, {}
        for j in range(8):
            t1h[j] = ftile(f"h1_{16 + j}_0")
            nc.vector.tensor_sub(t1h[j][:], h1[(j, 1)][:], h1[(j, 0)][:])
            t3h[j] = ftile(f"h1_{16 + j}_1")
            nc.vector.tensor_sub(t3h[j][:], h1[(8 + j, 1)][:], h1[(j, 1)][:])
        for j in range(8):
            t2h[j] = ftile(f"h1_{24 + j}_0")
            nc.vector.tensor_sub(t2h[j][:], h1[(8 + j, 1)][:], t1h[j][:])
            t4h[j] = ftile(f"h1_{24 + j}_1")
            nc.vector.tensor_sub(t4h[j][:], t2h[j][:], h1[(8 + j, 0)][:])

        def bsel1(i, j):
            if i == 0:
                return h1[(j, 0)][:]
            if i == 1:
                return h1[(8 + j, 0)][:]
            if i == 2:
                return h1[(8 + j, 1)][:]
            if i == 4:
                return t1h[j][:]
            if i == 5:
                return t2h[j][:]
            if i == 6:
                return t3h[j][:]
            return t4h[j][:]

        # fp8 merge: which weight variant and token half ride on which chain
        #   chain 1 (A12@B21) -> c11: +Wg,        tok 0   (variant 0)
        #   chain 2 (S4@B22)  -> c12: Wg - W16g,  tok 1   (variant 1)
        #   chain 3 (A22@T4)  -> c21: -W16g,      tok 0   (variant 2)
        #   chain 4 (S1@T1)   -> c22: +W16g,      tok 1   (variant 3)
        f8_merge = {1: (0, 0), 2: (1, 1), 3: (2, 0), 4: (3, 1)}

        h2 = {}
        h2_n = [0]

        def h2slot(st):
            i = h2_n[0]
            h2_n[0] += 1
            if i < 16:
                sl = xbig[i // 8][:, (i % 8) * TOK_TILE : (i % 8 + 1) * TOK_TILE]
            else:
                sl = ftile(f"h2_{i - 16}")[:]
            h2[st] = sl
            return sl

        def emit1(g, cname, sb, ps, is_sub):
            st = {"c11": (g, 0), "c21": (16 + g, 0), "c12": (g, 1),
                  "c22": (16 + g, 1)}[cname]
            c = spool.tile([P, TOK_TILE], bf, tag="st", name=f"c_{g}_{cname}")
            if is_sub:
                nc.vector.tensor_sub(c[:], sb[:], ps[:])
            else:
                nc.vector.tensor_add(c[:], sb[:], ps[:])
            ht = h2slot(st)
            nc.scalar.activation(ht, c[:], relu, scale=a1,
                                 bias=b1[:, st[0] : st[0] + 1])

        # ---- layer 1: hybrid 7-chain Winograd (bf16, K=0..2047) with fp8
        # DoubleRow partials (K=2048..4095) accumulated into chains 1-4.
        for g in range(16):
            pss = {}

            def chain(*idxs, g=g):
                tiles = []
                for i in idxs:
                    wA = wpool.tile([P, 512], bf, tag="w", name=f"l1w_{g}_{i}a")
                    wB = wpool.tile([P, 512], bf, tag="w", name=f"l1w_{g}_{i}b")
                    nc.sync.dma_start(out=wA[:], in_=w1w[i, g, :, 0:512])
                    nc.gpsimd.dma_start(out=wB[:], in_=w1w[i, g, :, 512:1024])
                    ps = pspool.tile([P, TOK_TILE], f32, tag="ps",
                                     name=f"ps1_{g}_{i}")
                    pss[i] = ps
                    tiles.append((i, wA, wB, ps))
                for j in range(8):
                    for i, wA, wB, ps in tiles:
                        w = wA if j < 4 else wB
                        jj = j if j < 4 else j - 4
                        nc.tensor.matmul(
                            ps[:],
                            w[:, jj * P : (jj + 1) * P],
                            bsel1(i, j),
                            start=(j == 0),
                            stop=(j == 7 and i not in f8_merge),
                        )
                for i, wA, wB, ps in tiles:
                    if i not in f8_merge:
                        continue
                    v, t = f8_merge[i]
                    for pr in range(8):
                        wf = fwpool.tile([P, 2, P], e4, tag="wf",
                                         name=f"l1f_{g}_{i}_{pr}")
                        nc.gpsimd.dma_start(out=wf[:], in_=w1f[v, g, pr])
                        nc.tensor.matmul(
                            ps[:],
                            wf[:],
                            f8[(pr, t)][:],
                            start=False,
                            stop=(pr == 7),
                            perf_mode=DR,
                        )

            # Mi chains: 0:B11 4:T1 6:T3 1:B21 2:B22 5:T2 3:T4
            chain(0, 4)
            m1s = mpool.tile([P, TOK_TILE], f32, tag="m1s", name=f"m1s_{g}")
            nc.scalar.copy(m1s[:], pss[0][:])
            chain(6, 1)
            emit1(g, "c11", m1s, pss[1], False)
            chain(2, 5)
            u2 = upool.tile([P, TOK_TILE], f32, tag="u2", name=f"u2_{g}")
            nc.vector.tensor_add(u2[:], m1s[:], pss[5][:])
            u4 = upool.tile([P, TOK_TILE], f32, tag="u4", name=f"u4_{g}")
            nc.vector.tensor_add(u4[:], u2[:], pss[4][:])
            u3 = upool.tile([P, TOK_TILE], f32, tag="u3", name=f"u3_{g}")
            nc.vector.tensor_add(u3[:], u2[:], pss[6][:])
            emit1(g, "c22", u3, pss[4], False)
            emit1(g, "c12", u4, pss[2], False)
            chain(3)
            emit1(g, "c21", u3, pss[3], True)

        # ---- layer 2 strassen (bf16, full K; no relu; outputs DMA'd as
        # they complete). T tiles reuse h1's slots (all dead after layer 1).
        t1b, t2b, t3b, t4b = {}, {}, {}, {}

        def tbuild2():
            for s in range(16):
                t1b[s] = ftile(f"h1_{s}_0")
                nc.vector.tensor_sub(t1b[s][:], h2[(s, 1)], h2[(s, 0)])
                t3b[s] = ftile(f"h1_{s}_1")
                nc.vector.tensor_sub(t3b[s][:], h2[(16 + s, 1)], h2[(s, 1)])
            for s in range(16):
                t2b[s] = ftile(f"h1_{16 + s}_0")
                nc.vector.tensor_sub(t2b[s][:], h2[(16 + s, 1)], t1b[s][:])
                t4b[s] = ftile(f"h1_{16 + s}_1")
                nc.vector.tensor_sub(t4b[s][:], t2b[s][:], h2[(16 + s, 0)])

        def bsel2(i, j):
            if i == 0:
                return h2[(j, 0)]
            if i == 1:
                return h2[(16 + j, 0)]
            if i == 2:
                return h2[(16 + j, 1)]
            if i == 4:
                return t1b[j][:]
            if i == 5:
                return t2b[j][:]
            if i == 6:
                return t3b[j][:]
            return t4b[j][:]

        def emit2(g, cname, sb, ps, is_sub):
            row, t = {"c11": (g, 0), "c21": (4 + g, 0), "c12": (g, 1),
                      "c22": (4 + g, 1)}[cname]
            o = opool.tile([P, TOK_TILE], bf, tag="o", name=f"o_{g}_{cname}")
            if is_sub:
                nc.vector.tensor_sub(o[:], sb[:], ps[:])
            else:
                nc.vector.tensor_add(o[:], sb[:], ps[:])
            q = nc.scalar if t == 0 else nc.gpsimd
            q.dma_start(
                out=outt[row * P : (row + 1) * P,
                         t * TOK_TILE : (t + 1) * TOK_TILE],
                in_=o[:],
            )

        tbuild2()
        for g in range(4):
            pss = {}

            def chain2(*idxs, g=g):
                tiles = []
                for i in idxs:
                    wA = wpool.tile([P, 1024], bf, tag="w2", name=f"w2_{g}_{i}a")
                    wB = wpool.tile([P, 1024], bf, tag="w2", name=f"w2_{g}_{i}b")
                    nc.sync.dma_start(out=wA[:], in_=w2s[i, g, :, 0:1024])
                    nc.gpsimd.dma_start(out=wB[:], in_=w2s[i, g, :, 1024:2048])
                    ps = pspool.tile([P, TOK_TILE], f32, tag="ps",
                                     name=f"ps2_{g}_{i}")
                    pss[i] = ps
                    tiles.append((i, wA, wB, ps))
                for j in range(16):
                    for i, wA, wB, ps in tiles:
                        w = wA if j < 8 else wB
                        jj = j if j < 8 else j - 8
                        nc.tensor.matmul(
                            ps[:],
                            w[:, jj * P : (jj + 1) * P],
                            bsel2(i, j),
                            start=(j == 0),
                            stop=(j == 15),
                        )

            chain2(0, 4)
            m1s = mpool.tile([P, TOK_TILE], f32, tag="m1s", name=f"m2s_{g}")
            nc.scalar.copy(m1s[:], pss[0][:])
            chain2(6, 1)
            emit2(g, "c11", m1s, pss[1], False)
            chain2(2, 5)
            u2 = upool.tile([P, TOK_TILE], f32, tag="u2", name=f"v2_{g}")
            nc.vector.tensor_add(u2[:], m1s[:], pss[5][:])
            u4 = upool.tile([P, TOK_TILE], f32, tag="u4", name=f"v4_{g}")
            nc.vector.tensor_add(u4[:], u2[:], pss[4][:])
            u3 = upool.tile([P, TOK_TILE], f32, tag="u3", name=f"v3_{g}")
            nc.vector.tensor_add(u3[:], u2[:], pss[6][:])
            emit2(g, "c22", u3, pss[4], False)
            emit2(g, "c12", u4, pss[2], False)
            chain2(3)
            emit2(g, "c21", u3, pss[3], True)

    _prune_dma_waits(nc)
    nc.finalize()
    return nc


def _pack_w(k):
    """Bool [K, N] -> bf16 +-1 packed [N/P, P, K]."""
    K, N = k.shape
    w = np.where(k, np.float32(1.0), np.float32(-1.0)).astype(BF16)
    return np.ascontiguousarray(
        w.reshape(K // P, P, N // P, P).transpose(2, 1, 0, 3).reshape(N // P, P, K)
    )


def _pack_f(w):
    """Float [K, N] -> bf16 packed [N/P, P, K] (strip, partition=K, free)."""
    K, N = w.shape
    w = np.ascontiguousarray(w).astype(BF16)
    return np.ascontiguousarray(
        w.reshape(K // P, P, N // P, P).transpose(2, 1, 0, 3).reshape(N // P, P, K)
    )


def _strassen_weights(kmat, alpha):
    """Weight-side Winograd combos for C = Wt @ H, Wt = (2k-1).T scaled by
    alpha (exact in bf16 for power-of-two alpha and small-int combos).
    Returns [7, M/2/P, P, K/2] with Mi order [A11, A12, S4, A22, S1, S2, S3]
    matching B order [B11, B21, B22, T4, T1, T2, T3]."""
    Wt = np.where(kmat, 1.0, -1.0).astype(np.float32).T
    M, K = Wt.shape
    mh, kh = M // 2, K // 2
    A11, A12 = Wt[:mh, :kh], Wt[:mh, kh:]
    A21, A22 = Wt[mh:, :kh], Wt[mh:, kh:]
    S1 = A21 + A22
    S2 = S1 - A11
    S3 = A11 - A21
    S4 = A12 - S2
    packs = []
    for A in (A11, A12, S4, A22, S1, S2, S3):
        packs.append(_pack_f((alpha * A).T))
    return np.ascontiguousarray(np.stack(packs))


def _pack_w1f(k1):
    """fp8 DoubleRow weight variants for L1's second contraction half.

    Returns [4, 16, 8, P, 2, P] e4m3 where variant v, group g, pair pr:
      v=0 (ride chain1 -> c11):  +W[g]
      v=1 (ride chain2 -> c12):  W[g] - W[16+g]
      v=2 (ride chain3 -> c21):  -W[16+g]
      v=3 (ride chain4 -> c22):  +W[16+g]
    with W[s] = sign(k1)[2048 + pr*256 + i*128 + k, s*128 + m]."""
    Wb = np.where(k1[2048:], np.float32(1.0), np.float32(-1.0))  # [2048, 4096]
    # [pr, i, k, s, m]
    Wr = Wb.reshape(8, 2, P, 32, P)
    out = np.zeros((4, 16, 8, P, 2, P), dtype=np.float32)
    for g in range(16):
        wg = Wr[:, :, :, g, :]          # [8, 2, 128, 128]
        w16 = Wr[:, :, :, 16 + g, :]
        for v, blk in ((0, wg), (1, wg - w16), (2, -w16), (3, w16)):
            out[v, g] = blk.transpose(0, 2, 1, 3)  # [8, k, i, m]
    return np.ascontiguousarray(out.astype(E4M3))


def _enable_ntff_trace():
    """Best-effort plumbing for trace=True under axon in this image."""
    import sys
    import types

    import concourse.bass_utils as bu

    bu.upload_artifacts = lambda tmpdir: tmpdir
    try:
        from antenv import axon_hooks
    except ImportError:
        import antenv

        axon_hooks = types.ModuleType("antenv.axon_hooks")
        _state = {"hook": None}
        axon_hooks.set_axon_ntff_profile_hook = lambda h: _state.__setitem__(
            "hook", h
        )
        axon_hooks.get_axon_ntff_profile_hook = lambda: _state["hook"]
        sys.modules["antenv.axon_hooks"] = axon_hooks
        antenv.axon_hooks = axon_hooks
    if axon_hooks.get_axon_ntff_profile_hook() is None:
        from trn_agent_boot.trn_boot import _ntff_profile_via_ctypes

        axon_hooks.set_axon_ntff_profile_hook(
            _ntff_profile_via_ctypes("/opt/axon/libaxon_pjrt.so")
        )


def kernel(x, k0, k1, k2, s0, s1, s2):
    global LAST_EXEC_TIME_NS, LAST_RESULT
    from concourse.bass_utils import run_bass_kernel_spmd

    if TRACE:
        _enable_ntff_trace()

    x = np.asarray(x)
    k1 = np.asarray(k1)
    a0 = 2.0 * float(np.asarray(s0))
    a1 = 2.0 * float(np.asarray(s1))
    a2 = float(np.asarray(s2))

    key = (a0, a1)
    if key not in _cache:
        _cache[key] = _build(a0, a1)
    nc = _cache[key]

    w0p = _pack_w(np.asarray(k0))
    w1wp = _strassen_weights(k1[:2048, :], 1.0)
    w1fp = _pack_w1f(k1)
    w2sp = _strassen_weights(np.asarray(k2), a2)
    # bias: a1 * mu1 * colsum over the fp8 half of W1, laid out [m, strip]
    colsum = np.where(k1[2048:], 1.0, -1.0).sum(axis=0)  # [4096]
    b1 = np.empty((P, 33), dtype=np.float32)
    b1[:, :32] = (a1 * MU1 * colsum).reshape(32, P).T
    b1[:, 32] = -MU1
    b1 = np.ascontiguousarray(b1)

    in_maps = []
    for i in range(N_CORES):
        xs = x[i * TOK_PER_CORE : (i + 1) * TOK_PER_CORE].astype(BF16)
        xsT = np.ascontiguousarray(xs.T)  # [feat, tok]
        xp = np.ascontiguousarray(
            xsT.reshape(D_IN // P, P, NT, TOK_TILE)
            .transpose(2, 1, 0, 3)
            .reshape(NT, P, D_IN // P * TOK_TILE)
        )
        in_maps.append(
            {
                "xt": xp,
                "w0p": w0p,
                "w1w": w1wp,
                "w1f": w1fp,
                "b1t": b1,
                "w2s": w2sp,
            }
        )

    res = run_bass_kernel_spmd(
        nc, in_maps, list(range(N_CORES)), trace=TRACE, trace_cores=TRACE_CORES
    )
    LAST_EXEC_TIME_NS = res.exec_time_ns
    LAST_RESULT = res
    out = np.concatenate(
        [res.results[i]["outt"].T.astype(np.float32) for i in range(N_CORES)],
        axis=0
    )
    return np.ascontiguousarray(out)


# revision 16
# speedup vs baseline: 1.5518x; 1.5518x over previous
"""Trainium2 Bass kernel for a 3-layer binary-weight MLP.

Problem (nn_MLP_56779467653689):
    x: [8192, 1024] f32
    h = relu(s0 * (x @ W0)) * 2      W0 = 2*k0-1  in {-1,+1}, [1024, 4096]
    h = relu(s1 * (h @ W1)) * 2      W1 [4096, 4096]
    out = s2 * (h @ W2)              W2 [4096, 1024]

Strategy: data-parallel over tokens across 8 NeuronCores (1024/core),
activations [features, tokens] in SBUF.

Layer 1 is hybrid-precision: the first 2048 contraction features run one
level of Winograd-Strassen in bf16 (7/8 multiplies); the other 2048 run
naive fp8(e4m3) matmuls in DoubleRow perf mode (2 contraction rows per PE
cell per cycle).  The fp8 activations are mean-removed (hardcoded mu,
exactly compensated by a per-output-feature bias at eviction) to shrink
quantization error.  The fp8 partial sums accumulate directly into the
Winograd chains' PSUM banks using host-precomputed weight variants
(c11:+Wg, c12:Wg-W16g, c21:-W16g, c22:+W16g, all exact in e4m3), so the
Winograd combine algebra routes them to the right output quadrant with no
extra vector work.

Layer 2 uses one level of bf16 Winograd-Strassen over the full
contraction (fp8 there would blow the error budget).  Layer 0 is naive
bf16.  PE multiply count: L0 8/8, L1 (7/8)/2 + (1/2)/2-rate, L2 7/8.
"""

from contextlib import ExitStack

import ml_dtypes
import numpy as np

P = 128
TOKENS = 8192
D_IN = 1024
D_H = 4096
D_OUT = 1024
N_CORES = 8
TOK_PER_CORE = TOKENS // N_CORES  # 1024
TOK_TILE = 512
NT = TOK_PER_CORE // TOK_TILE  # 2

MU1 = 0.7975461096539505  # approx mean of h1; any value is corrected exactly

BF16 = ml_dtypes.bfloat16
E4M3 = ml_dtypes.float8_e4m3

TRACE = False
TRACE_CORES = None
LAST_EXEC_TIME_NS = None
LAST_RESULT = None

_cache = {}


def _prune_dma_waits(nc, max_waits=1):
    """Drop transitively-implied waits from DMA instructions.

    DMA queue-entry descriptors hold a single sync wait; Tile's sem
    assignment is per-proc minimal but not transitively minimal across
    procs, so a recycled SBUF slot's DMA can carry WAR (engine) + WAW
    (prev slot writer's DMA lane) + lane-recycle waits = 3. The WAW (and
    often the recycle) wait is implied by the engine wait: the readers
    counted by the WAR threshold themselves waited on those DMAs.

    Soundness: a wait (s >= v) on instruction I is dropped only when the
    completion clocks implied by I's *other* waits already guarantee
    cumulative increments of s reached v. Completion clocks are built
    forward over the scheduled BIR order giving same-stream predecessor
    credit only to in-order engines (PE/ACT/DVE/SP), never to DMA lanes
    or Pool. Unrecognized wait/update modes contribute no credit, so
    unknowns can only inhibit pruning, never enable it.
    """
    import bisect

    import bass_rust

    IN_ORDER_ENGINES = {
        "EngineType.PE",
        "EngineType.Activation",
        "EngineType.DVE",
        "EngineType.SP",
    }

    sem_hist = {}
    sem_cum = {}
    eng_clock = {}
    poisoned = set()

    def cc(sem, val):
        if sem in poisoned:
            return None
        hist = sem_hist.get(sem)
        if not hist or hist[0][-1] < val:
            return None
        return hist[1][bisect.bisect_left(hist[0], val)]

    def merge(dst, src):
        for k, v in src.items():
            if dst.get(k, 0) < v:
                dst[k] = v

    pruned = 0
    for bb in nc.m.functions[0].blocks:
        for inst in bb.instructions:
            si = inst.sync_info
            waits = list(si.on_wait or []) if si is not None else []
            ups = list(si.on_update or []) if si is not None else []
            is_dma = type(inst).__name__ == "InstDMACopy"

            clock = {}
            if not is_dma:
                prev = eng_clock.get(str(inst.engine))
                if prev is not None and str(inst.engine) in IN_ORDER_ENGINES:
                    merge(clock, prev)
            for w in waits:
                if w.wait_mode == "sem-ge-imm" and w.wait_value is not None:
                    c = cc(w.ant_name, w.wait_value)
                    if c is not None:
                        merge(clock, c)

            tname = type(inst).__name__
            if is_dma:
                cap = max_waits
            elif tname in ("InstDrain", "InstEventSemaphore", "InstCall",
                           "InstUnconditionalBranch", "InstISA"):
                cap = None
            else:
                cap = 2

            if cap is not None and len(waits) > cap:
                kept = list(waits)
                changed = True
                while len(kept) > cap and changed:
                    changed = False
                    for w in list(kept):
                        if w.wait_mode != "sem-ge-imm" or w.wait_value is None:
                            continue
                        implied = {}
                        provable = True
                        for o in kept:
                            if o is w:
                                continue
                            if o.wait_mode != "sem-ge-imm" or o.wait_value is None:
                                provable = False
                                break
                            c = cc(o.ant_name, o.wait_value)
                            if c is None:
                                provable = False
                                break
                            merge(implied, c)
                        if provable and implied.get(w.ant_name, 0) >= w.wait_value:
                            kept.remove(w)
                            pruned += 1
                            changed = True
                            break
                if len(kept) != len(waits):
                    inst.sync_info = bass_rust.SyncInfo(on_wait=kept, on_update=ups)

            own = {}
            for u in ups:
                if u.update_mode not in ("sem-inc", "sem-add-imm"):
                    poisoned.add(u.ant_name)
                    continue
                inc = 1 if u.update_mode == "sem-inc" else u.update_value
                if inc is None:
                    poisoned.add(u.ant_name)
                    continue
                sem = u.ant_name
                sem_cum[sem] = sem_cum.get(sem, 0) + inc
                own[sem] = sem_cum[sem]
            merge(clock, own)
            for sem, cum in own.items():
                vals, clocks = sem_hist.setdefault(sem, ([], []))
                vals.append(cum)
                clocks.append(clock)
            if not is_dma:
                eng_clock[str(inst.engine)] = clock
    return pruned


def _build(a0, a1):
    """SPMD single-core program. L2's eviction scale is folded into its
    host-packed Strassen combos; a0/a1 are applied at eviction."""
    import concourse.mybir as mybir
    import concourse.tile as tile
    from concourse import bacc

    nc = bacc.Bacc(
        "TRN2",
        target_bir_lowering=False,
        debug=False,
        enable_asserts=False,
        num_devices=N_CORES,
    )
    bf = mybir.dt.bfloat16
    f32 = mybir.dt.float32
    e4 = mybir.dt.float8e4
    DR = mybir.MatmulPerfMode.DoubleRow

    xt = nc.dram_tensor("xt", [NT, P, D_IN // P * TOK_TILE], bf,
                        kind="ExternalInput")
    w0p = nc.dram_tensor("w0p", [D_H // P, P, D_IN], bf, kind="ExternalInput")
    w1w = nc.dram_tensor("w1w", [7, 16, P, 1024], bf, kind="ExternalInput")
    w1f = nc.dram_tensor("w1f", [4, 16, P, 8, 2, P], e4, kind="ExternalInput")
    b1t = nc.dram_tensor("b1t", [P, 33], f32, kind="ExternalInput")
    w2s = nc.dram_tensor("w2s", [7, 4, P, D_H // 2], bf, kind="ExternalInput")
    outt = nc.dram_tensor("outt", [D_OUT, TOK_PER_CORE], bf, kind="ExternalOutput")

    relu = mybir.ActivationFunctionType.Relu
    ident = mybir.ActivationFunctionType.Identity

    with tile.TileContext(nc) as tc, ExitStack() as ctx:
        flat = ctx.enter_context(tc.tile_pool(name="flat", bufs=1))
        wpool = ctx.enter_context(tc.tile_pool(name="wp", bufs=8))
        fwpool = ctx.enter_context(tc.tile_pool(name="fwp", bufs=4))
        upool = ctx.enter_context(tc.tile_pool(name="up", bufs=1))
        mpool = ctx.enter_context(tc.tile_pool(name="mp", bufs=2))
        spool = ctx.enter_context(tc.tile_pool(name="sp", bufs=4))
        opool = ctx.enter_context(tc.tile_pool(name="op", bufs=4))
        pspool = ctx.enter_context(tc.tile_pool(name="psp", bufs=8, space="PSUM"))

        def ftile(tag):
            return flat.tile([P, TOK_TILE], bf, tag=tag, name=tag)

        # ---- layer 0: naive. x is host-packed to [t, 128, j*512+c] so each
        # half loads as wide DMAs split across queues.
        xbig = []
        for t in range(NT):
            xb = flat.tile([P, D_IN // P * TOK_TILE], bf, tag=f"xb_{t}",
                           name=f"xb_{t}")
            for j in range(D_IN // P):
                q = nc.scalar if j % 2 == 0 else nc.gpsimd
                q.dma_start(
                    out=xb[:, j * TOK_TILE : (j + 1) * TOK_TILE],
                    in_=xt[t, :, j * TOK_TILE : (j + 1) * TOK_TILE],
                )
            xbig.append(xb)

        def x_sl(j, t):
            return xbig[t][:, j * TOK_TILE : (j + 1) * TOK_TILE]

        # bias vector for L1 eviction (per out-feature, column n = strip n);
        # column 32 holds -mu1 for the fp8 mean-removal cast. Loaded after x
        # so it doesn't delay the first layer-0 chains.
        b1 = flat.tile([P, 33], f32, tag="b1", name="b1")
        nc.sync.dma_start(out=b1[:], in_=b1t[:, :])

        h1 = {}
        w0tiles = {}
        # fp8 copies of h1 strips 16..31 (mean-removed), paired for DoubleRow
        f8 = {}
        for pr in range(8):
            for t in range(NT):
                f8[(pr, t)] = flat.tile([P, 2, TOK_TILE], e4,
                                        tag=f"f8_{pr}_{t}", name=f"f8_{pr}_{t}")

        def l0_chain(n, t):
            if n not in w0tiles:
                w = wpool.tile([P, D_IN], bf, tag="w", name=f"w0_{n}")
                nc.sync.dma_start(out=w[:], in_=w0p[n])
                w0tiles[n] = w
            w = w0tiles[n]
            ps = pspool.tile([P, TOK_TILE], f32, tag="ps", name=f"ps0_{n}_{t}")
            for j in range(D_IN // P):
                nc.tensor.matmul(
                    ps[:],
                    w[:, j * P : (j + 1) * P],
                    x_sl(j, t),
                    start=(j == 0),
                    stop=(j == D_IN // P - 1),
                )
            ht = ftile(f"h1_{n}_{t}")
            nc.scalar.activation(ht[:], ps[:], relu, scale=a0)
            h1[(n, t)] = ht
            if n >= 16:
                # fp8 copy for L1's DoubleRow half: (h - mu) cast to e4m3
                s = n - 16
                nc.scalar.activation(
                    f8[(s // 2, t)][:, s % 2, :], ht[:], ident,
                    bias=b1[:, 32:33],
                )

        # First strips run token-half 0 only, buying arrival slack for the
        # second half of x; then the normal (n, t) sweep.
        l0_order = [(n, 0) for n in range(4)] + [(n, 1) for n in range(4)]
        l0_order += [(n, t) for n in range(4, D_H // P) for t in range(NT)]
        for n, t in l0_order:
            l0_chain(n, t)

        # ---- L1 prep: T-combos for the bf16-Winograd half (strips 0..15),
        # written into the now-dead strips 16..31 slots. All reads are from
        # strips 0..15, so they never race the fp8 casts above.
        t1h, t2h, t3h, t4h = {}, {}, {}, {}
        for j in range(8):
            t1h[j] = ftile(f"h1_{16 + j}_0")
            nc.vector.tensor_sub(t1h[j][:], h1[(j, 1)][:], h1[(j, 0)][:])
            t3h[j] = ftile(f"h1_{16 + j}_1")
            nc.vector.tensor_sub(t3h[j][:], h1[(8 + j, 1)][:], h1[(j, 1)][:])
        for j in range(8):
            t2h[j] = ftile(f"h1_{24 + j}_0")
            nc.vector.tensor_sub(t2h[j][:], h1[(8 + j, 1)][:], t1h[j][:])
            t4h[j] = ftile(f"h1_{24 + j}_1")
            nc.vector.tensor_sub(t4h[j][:], t2h[j][:], h1[(8 + j, 0)][:])

        def bsel1(i, j):
            if i == 0:
                return h1[(j, 0)][:]
            if i == 1:
                return h1[(8 + j, 0)][:]
            if i == 2:
                return h1[(8 + j, 1)][:]
            if i == 4:
                return t1h[j][:]
            if i == 5:
                return t2h[j][:]
            if i == 6:
                return t3h[j][:]
            return t4h[j][:]

        # fp8 merge: which weight variant and token half ride on which chain
        #   chain 1 (A12@B21) -> c11: +Wg,        tok 0   (variant 0)
        #   chain 2 (S4@B22)  -> c12: Wg - W16g,  tok 1   (variant 1)
        #   chain 3 (A22@T4)  -> c21: -W16g,      tok 0   (variant 2)
        #   chain 4 (S1@T1)   -> c22: +W16g,      tok 1   (variant 3)
        f8_merge = {1: (0, 0), 2: (1, 1), 3: (2, 0), 4: (3, 1)}

        h2 = {}
        h2_n = [0]

        def h2slot(st):
            i = h2_n[0]
            h2_n[0] += 1
            if i < 16:
                sl = xbig[i // 8][:, (i % 8) * TOK_TILE : (i % 8 + 1) * TOK_TILE]
            else:
                sl = ftile(f"h2_{i - 16}")[:]
            h2[st] = sl
            return sl

        def emit1(g, cname, sb, ps, is_sub):
            st = {"c11": (g, 0), "c21": (16 + g, 0), "c12": (g, 1),
                  "c22": (16 + g, 1)}[cname]
            c = spool.tile([P, TOK_TILE], bf, tag="st", name=f"c_{g}_{cname}")
            if is_sub:
                nc.vector.tensor_sub(c[:], sb[:], ps[:])
            else:
                nc.vector.tensor_add(c[:], sb[:], ps[:])
            ht = h2slot(st)
            nc.scalar.activation(ht, c[:], relu, scale=a1,
                                 bias=b1[:, st[0] : st[0] + 1])

        # ---- layer 1: hybrid 7-chain Winograd (bf16, K=0..2047) with fp8
        # DoubleRow partials (K=2048..4095) accumulated into chains 1-4.
        for g in range(16):
            pss = {}

            def chain(*idxs, g=g):
                tiles = []
                for i in idxs:
                    wA = wpool.tile([P, 512], bf, tag="w", name=f"l1w_{g}_{i}a")
                    wB = wpool.tile([P, 512], bf, tag="w", name=f"l1w_{g}_{i}b")
                    nc.sync.dma_start(out=wA[:], in_=w1w[i, g, :, 0:512])
                    nc.gpsimd.dma_start(out=wB[:], in_=w1w[i, g, :, 512:1024])
                    ps = pspool.tile([P, TOK_TILE], f32, tag="ps",
                                     name=f"ps1_{g}_{i}")
                    pss[i] = ps
                    tiles.append((i, wA, wB, ps))
                for j in range(8):
                    for i, wA, wB, ps in tiles:
                        w = wA if j < 4 else wB
                        jj = j if j < 4 else j - 4
                        nc.tensor.matmul(
                            ps[:],
                            w[:, jj * P : (jj + 1) * P],
                            bsel1(i, j),
                            start=(j == 0),
                            stop=(j == 7 and i not in f8_merge),
                        )
                for i, wA, wB, ps in tiles:
                    if i not in f8_merge:
                        continue
                    v, t = f8_merge[i]
                    wf = fwpool.tile([P, 8, 2, P], e4, tag="wf",
                                     name=f"l1f_{g}_{i}")
                    nc.gpsimd.dma_start(out=wf[:], in_=w1f[v, g])
                    for pr in range(8):
                        nc.tensor.matmul(
                            ps[:],
                            wf[:, pr, :, :],
                            f8[(pr, t)][:],
                            start=False,
                            stop=(pr == 7),
                            perf_mode=DR,
                        )

            # Mi chains: 0:B11 4:T1 6:T3 1:B21 2:B22 5:T2 3:T4
            chain(0, 4)
            m1s = mpool.tile([P, TOK_TILE], f32, tag="m1s", name=f"m1s_{g}")
            nc.scalar.copy(m1s[:], pss[0][:])
            chain(6, 1)
            emit1(g, "c11", m1s, pss[1], False)
            chain(2, 5)
            u2 = upool.tile([P, TOK_TILE], f32, tag="u2", name=f"u2_{g}")
            nc.vector.tensor_add(u2[:], m1s[:], pss[5][:])
            u4 = upool.tile([P, TOK_TILE], f32, tag="u4", name=f"u4_{g}")
            nc.vector.tensor_add(u4[:], u2[:], pss[4][:])
            u3 = upool.tile([P, TOK_TILE], f32, tag="u3", name=f"u3_{g}")
            nc.vector.tensor_add(u3[:], u2[:], pss[6][:])
            emit1(g, "c22", u3, pss[4], False)
            emit1(g, "c12", u4, pss[2], False)
            chain(3)
            emit1(g, "c21", u3, pss[3], True)

        # ---- layer 2 strassen (bf16, full K; no relu; outputs DMA'd as
        # they complete). T tiles reuse h1's slots (all dead after layer 1).
        t1b, t2b, t3b, t4b = {}, {}, {}, {}

        def tbuild2():
            for s in range(16):
                t1b[s] = ftile(f"h1_{s}_0")
                nc.vector.tensor_sub(t1b[s][:], h2[(s, 1)], h2[(s, 0)])
                t3b[s] = ftile(f"h1_{s}_1")
                nc.vector.tensor_sub(t3b[s][:], h2[(16 + s, 1)], h2[(s, 1)])
            for s in range(16):
                t2b[s] = ftile(f"h1_{16 + s}_0")
                nc.vector.tensor_sub(t2b[s][:], h2[(16 + s, 1)], t1b[s][:])
                t4b[s] = ftile(f"h1_{16 + s}_1")
                nc.vector.tensor_sub(t4b[s][:], t2b[s][:], h2[(16 + s, 0)])

        def bsel2(i, j):
            if i == 0:
                return h2[(j, 0)]
            if i == 1:
                return h2[(16 + j, 0)]
            if i == 2:
                return h2[(16 + j, 1)]
            if i == 4:
                return t1b[j][:]
            if i == 5:
                return t2b[j][:]
            if i == 6:
                return t3b[j][:]
            return t4b[j][:]

        def emit2(g, cname, sb, ps, is_sub):
            row, t = {"c11": (g, 0), "c21": (4 + g, 0), "c12": (g, 1),
                      "c22": (4 + g, 1)}[cname]
            o = opool.tile([P, TOK_TILE], bf, tag="o", name=f"o_{g}_{cname}")
            if is_sub:
                nc.vector.tensor_sub(o[:], sb[:], ps[:])
            else:
                nc.vector.tensor_add(o[:], sb[:], ps[:])
            q = nc.scalar if t == 0 else nc.gpsimd
            q.dma_start(
                out=outt[row * P : (row + 1) * P,
                         t * TOK_TILE : (t + 1) * TOK_TILE],
                in_=o[:],
            )

        tbuild2()
        for g in range(4):
            pss = {}

            def chain2(*idxs, g=g):
                tiles = []
                for i in idxs:
                    wA = wpool.tile([P, 1024], bf, tag="w2", name=f"w2_{g}_{i}a")
                    wB = wpool.tile([P, 1024], bf, tag="w2", name=f"w2_{g}_{i}b")
                    nc.sync.dma_start(out=wA[:], in_=w2s[i, g, :, 0:1024])
                    nc.gpsimd.dma_start(out=wB[:], in_=w2s[i, g, :, 1024:2048])
                    ps = pspool.tile([P, TOK_TILE], f32, tag="ps",
                                     name=f"ps2_{g}_{i}")
                    pss[i] = ps
                    tiles.append((i, wA, wB, ps))
                for j in range(16):
                    for i, wA, wB, ps in tiles:
                        w = wA if j < 8 else wB
                        jj = j if j < 8 else j - 8
                        nc.tensor.matmul(
                            ps[:],
                            w[:, jj * P : (jj + 1) * P],
                            bsel2(i, j),
                            start=(j == 0),
                            stop=(j == 15),
                        )

            chain2(0, 4)
            m1s = mpool.tile([P, TOK_TILE], f32, tag="m1s", name=f"m2s_{g}")
            nc.scalar.copy(m1s[:], pss[0][:])
            chain2(6, 1)
            emit2(g, "c11", m1s, pss[1], False)
            chain2(2, 5)
            u2 = upool.tile([P, TOK_TILE], f32, tag="u2", name=f"v2_{g}")
            nc.vector.tensor_add(u2[:], m1s[:], pss[5][:])
            u4 = upool.tile([P, TOK_TILE], f32, tag="u4", name=f"v4_{g}")
            nc.vector.tensor_add(u4[:], u2[:], pss[4][:])
            u3 = upool.tile([P, TOK_TILE], f32, tag="u3", name=f"v3_{g}")
            nc.vector.tensor_add(u3[:], u2[:], pss[6][:])
            emit2(g, "c22", u3, pss[4], False)
            emit2(g, "c12", u4, pss[2], False)
            chain2(3)
            emit2(g, "c21", u3, pss[3], True)

    _prune_dma_waits(nc)
    nc.finalize()
    return nc


def _pack_w(k):
    """Bool [K, N] -> bf16 +-1 packed [N/P, P, K]."""
    K, N = k.shape
    w = np.where(k, np.float32(1.0), np.float32(-1.0)).astype(BF16)
    return np.ascontiguousarray(
        w.reshape(K // P, P, N // P, P).transpose(2, 1, 0, 3).reshape(N // P, P, K)
    )


def _pack_f(w):
    """Float [K, N] -> bf16 packed [N/P, P, K] (strip, partition=K, free)."""
    K, N = w.shape
    w = np.ascontiguousarray(w).astype(BF16)
    return np.ascontiguousarray(
        w.reshape(K // P, P, N // P, P).transpose(2, 1, 0, 3).reshape(N // P, P, K)
    )


def _strassen_weights(kmat, alpha):
    """Weight-side Winograd combos for C = Wt @ H, Wt = (2k-1).T scaled by
    alpha (exact in bf16 for power-of-two alpha and small-int combos).
    Returns [7, M/2/P, P, K/2] with Mi order [A11, A12, S4, A22, S1, S2, S3]
    matching B order [B11, B21, B22, T4, T1, T2, T3]."""
    Wt = np.where(kmat, 1.0, -1.0).astype(np.float32).T
    M, K = Wt.shape
    mh, kh = M // 2, K // 2
    A11, A12 = Wt[:mh, :kh], Wt[:mh, kh:]
    A21, A22 = Wt[mh:, :kh], Wt[mh:, kh:]
    S1 = A21 + A22
    S2 = S1 - A11
    S3 = A11 - A21
    S4 = A12 - S2
    packs = []
    for A in (A11, A12, S4, A22, S1, S2, S3):
        packs.append(_pack_f((alpha * A).T))
    return np.ascontiguousarray(np.stack(packs))


def _pack_w1f(k1):
    """fp8 DoubleRow weight variants for L1's second contraction half.

    Returns [4, 16, P, 8, 2, P] e4m3 where variant v, group g, pair pr:
      v=0 (ride chain1 -> c11):  +W[g]
      v=1 (ride chain2 -> c12):  W[g] - W[16+g]
      v=2 (ride chain3 -> c21):  -W[16+g]
      v=3 (ride chain4 -> c22):  +W[16+g]
    with W[s] = sign(k1)[2048 + pr*256 + i*128 + k, s*128 + m]."""
    Wb = np.where(k1[2048:], np.float32(1.0), np.float32(-1.0))  # [2048, 4096]
    # [pr, i, k, s, m]
    Wr = Wb.reshape(8, 2, P, 32, P)
    out = np.zeros((4, 16, P, 8, 2, P), dtype=np.float32)
    for g in range(16):
        wg = Wr[:, :, :, g, :]          # [8, 2, 128, 128]
        w16 = Wr[:, :, :, 16 + g, :]
        for v, blk in ((0, wg), (1, wg - w16), (2, -w16), (3, w16)):
            out[v, g] = blk.transpose(2, 0, 1, 3)  # [k, pr, i, m]
    return np.ascontiguousarray(out.astype(E4M3))


def _enable_ntff_trace():
    """Best-effort plumbing for trace=True under axon in this image."""
    import sys
    import types

    import concourse.bass_utils as bu

    bu.upload_artifacts = lambda tmpdir: tmpdir
    try:
        from antenv import axon_hooks
    except ImportError:
        import antenv

        axon_hooks = types.ModuleType("antenv.axon_hooks")
        _state = {"hook": None}
        axon_hooks.set_axon_ntff_profile_hook = lambda h: _state.__setitem__(
            "hook", h
        )
        axon_hooks.get_axon_ntff_profile_hook = lambda: _state["hook"]
        sys.modules["antenv.axon_hooks"] = axon_hooks
        antenv.axon_hooks = axon_hooks
    if axon_hooks.get_axon_ntff_profile_hook() is None:
        from trn_agent_boot.trn_boot import _ntff_profile_via_ctypes

        axon_hooks.set_axon_ntff_profile_hook(
            _ntff_profile_via_ctypes("/opt/axon/libaxon_pjrt.so")
        )


def kernel(x, k0, k1, k2, s0, s1, s2):
    global LAST_EXEC_TIME_NS, LAST_RESULT
    from concourse.bass_utils import run_bass_kernel_spmd

    if TRACE:
        _enable_ntff_trace()

    x = np.asarray(x)
    k1 = np.asarray(k1)
    a0 = 2.0 * float(np.asarray(s0))
    a1 = 2.0 * float(np.asarray(s1))
    a2 = float(np.asarray(s2))

    key = (a0, a1)
    if key not in _cache:
        _cache[key] = _build(a0, a1)
    nc = _cache[key]

    w0p = _pack_w(np.asarray(k0))
    w1wp = _strassen_weights(k1[:2048, :], 1.0)
    w1fp = _pack_w1f(k1)
    w2sp = _strassen_weights(np.asarray(k2), a2)
    # bias: a1 * mu1 * colsum over the fp8 half of W1, laid out [m, strip]
    colsum = np.where(k1[2048:], 1.0, -1.0).sum(axis=0)  # [4096]
    b1 = np.empty((P, 33), dtype=np.float32)
    b1[:, :32] = (a1 * MU1 * colsum).reshape(32, P).T
    b1[:, 32] = -MU1
    b1 = np.ascontiguousarray(b1)

    in_maps = []
    for i in range(N_CORES):
        xs = x[i * TOK_PER_CORE : (i + 1) * TOK_PER_CORE].astype(BF16)
        xsT = np.ascontiguousarray(xs.T)  # [feat, tok]
        xp = np.ascontiguousarray(
            xsT.reshape(D_IN // P, P, NT, TOK_TILE)
            .transpose(2, 1, 0, 3)
            .reshape(NT, P, D_IN // P * TOK_TILE)
        )
        in_maps.append(
            {
                "xt": xp,
                "w0p": w0p,
                "w1w": w1wp,
                "w1f": w1fp,
                "b1t": b1,
                "w2s": w2sp,
            }
        )

    res = run_bass_kernel_spmd(
        nc, in_maps, list(range(N_CORES)), trace=TRACE, trace_cores=TRACE_CORES
    )
    LAST_EXEC_TIME_NS = res.exec_time_ns
    LAST_RESULT = res
    out = np.concatenate(
        [res.results[i]["outt"].T.astype(np.float32) for i in range(N_CORES)],
        axis=0
    )
    return np.ascontiguousarray(out)
